# revision 1
# baseline (speedup 1.0000x reference)
"""Trainium2 Bass kernel for causal multi-head attention with RoPE.

nn_CausalAttention: x [2, 2048, 2048], Wq/Wk/Wv [2048, 2048] (y = x @ W.T),
16 heads of dim 128, RoPE, causal fp32 softmax.

Sharding (tensor-parallel heads, per the problem hint): each of the 8
NeuronCores owns 2 heads (a 256-wide slice of the QKV output dim) for both
batch elements. Each core runs the full pipeline for its heads; the full
output is assembled on host by concatenating per-core feature slices (no
collectives needed).

Per-core kernel (Bass/Tile, float32r matmuls at full PE rate):
  Phase A (per batch): q^T/k^T/v^T in [head_dim x seq] layout from a
    host-pre-transposed x^T with 512-wide moving operands; RoPE is fused into
    the PSUM->SBUF eviction using a host-side row permutation of Wq/Wk
    (quadrant-16 rotate-half layout) so the pair-combine is a single DVE
    stream_shuffle; v^T is PE-transposed into [seq x head_dim] tiles.
  Phase B (per batch, per head): causal attention in transposed-score layout
    S^T = K-tile^T^T @ q^T (keys on partitions, queries on the free dim), exp
    on the scalar engine with the 1/sqrt(d) scale fused, tile-level causality
    (upper-triangle key tiles skipped, diagonal tiles sub-ranged), a single
    128x128 triangular mask applied post-exp on the diagonal window, softmax
    denominator accumulated with an all-ones matmul broadcast across PSUM
    partitions, fast approximate reciprocal, normalization fused into the
    output eviction. Output is written head-dim-major and untransposed on the
    host during the gather.
"""

import math

import numpy as np

import concourse.bacc as bacc
import concourse.bass as bass
import concourse.mybir as mybir
import concourse.tile as tile
from concourse import bass_utils

F32 = mybir.dt.float32
F32R = mybir.dt.float32r
AF = mybir.ActivationFunctionType

S = 2048
M = 2048
NCORES = 8

D = 128          # head dim
NH = 2           # heads per core
NB = 2           # batches
SE = 256         # phase-A sequence slab ("eighth" at S=2048)
QT = 512         # phase-B query tile


def _rope_perm(n):
    """Row permutation for the quadrant-16 RoPE layout.

    New row p (within a 128-row head block): quadrant qd = p//32, r = p%32.
    r < 16  -> even element of pair i = 16*qd + r      (old row 2i)
    r >= 16 -> odd  element of pair i = 16*qd + (r-16) (old row 2i+1)
    Pair elements are 16 partitions apart inside one 32-partition quadrant,
    so the RoPE combine is a stream_shuffle with a 16-rotation mask.
    """
    perm = []
    for hb in range(n // D):
        base = hb * D
        for qd in range(4):
            perm += [base + 2 * (16 * qd + r) for r in range(16)]
            perm += [base + 2 * (16 * qd + r) + 1 for r in range(16)]
    return np.array(perm)


SWAP16 = [(i + 16) % 32 for i in range(32)]


def prep_core_inputs(x, Wq, Wk, Wv, core, S, M):
    """Host-side shard prep for one core. x [2,S,M], W* [M', M] where
    rows [core*256, core*256+256) of W* are this core's heads."""
    nsl = slice(core * NH * D, (core + 1) * NH * D)
    perm = _rope_perm(NH * D)
    wq = Wq[nsl][perm]
    wk = Wk[nsl][perm]
    wv = Wv[nsl]

    theta = np.exp(
        -np.float32(np.log(10000.0))
        * (np.arange(0, D, 2, dtype=np.float32) / np.float32(D))
    ).astype(np.float32)
    pos = np.arange(S, dtype=np.float32)
    freqs = theta[:, None] * pos[None, :]  # [64, S], row i = theta_i * s
    cos_t, sin_t = np.cos(freqs), np.sin(freqs)
    # quadrant-16 layout: partition p -> pair i(p) = 16*(p//32) + (p%16)
    p = np.arange(128)
    i_of_p = 16 * (p // 32) + (p % 16)
    is_odd = (p % 32) >= 16
    packC = cos_t[i_of_p].astype(np.float32)                    # [128, S]
    packS = np.where(
        is_odd[:, None], -sin_t[i_of_p], sin_t[i_of_p]
    ).astype(np.float32)

    kk, qq = np.meshgrid(np.arange(128), np.arange(128), indexing="ij")
    tri = (kk <= qq).astype(np.float32)

    return {
        "xT0": np.ascontiguousarray(x[0].T),
        "xT1": np.ascontiguousarray(x[1].T),
        "wqT": np.ascontiguousarray(wq.T),
        "wkT": np.ascontiguousarray(wk.T),
        "wvT": np.ascontiguousarray(wv.T),
        "packC": packC,
        "packS": packS,
        "tri": tri,
        "ones": np.ones((128, 128), dtype=np.float32),
        "ident": np.eye(128, dtype=np.float32),
    }


def build_attention(tc: tile.TileContext, io: dict, S: int, M: int, rdt=F32R):
    """v2: 512-wide phase-A slabs, per-batch phase split, diagonal
    sub-ranging in phase B, fast approx reciprocal."""
    nc = tc.nc
    MC = M // 128          # m chunks
    SLAB = 512
    NE = S // SLAB         # phase-A slabs per batch
    NQT = S // QT          # phase-B query tiles
    NST = S // 128         # 128-row seq tiles per batch
    scale = 1.0 / math.sqrt(D)

    xT = [io["xT0"], io["xT1"]]
    outT = io["outT"]

    with (
        tc.tile_pool(name="wpool", bufs=1) as wpool,
        tc.tile_pool(name="constpool", bufs=1) as constpool,
        tc.tile_pool(name="xp", bufs=2) as xpool,
        tc.tile_pool(name="rope", bufs=1) as ropetmp,
        tc.tile_pool(name="vtp", bufs=4) as vtpool,
        tc.tile_pool(name="pack", bufs=1) as packpool,
    ):
        w_sb = {}
        for name in ("wqT", "wkT", "wvT"):
            w = wpool.tile([128, MC, NH * D], rdt, tag=name, name=name)
            w_sb[name] = w
        tri_sb = constpool.tile([128, 128], rdt)
        ones_sb = constpool.tile([128, 128], rdt)
        ident_sb = constpool.tile([128, 128], rdt)
        # (DMAs for tri/ones are issued inside phase B; ident inside phase A
        #  after the first slab so they don't delay the critical first loads)

        for b in range(NB):
            with tc.tile_pool(name=f"qkv{b}", bufs=1) as qkvp:
                qT_sb = qkvp.tile([128, NH, S], rdt, name="qT_sb")
                kT_sb = qkvp.tile([128, NH, S], rdt, name="kT_sb")
                v_sb = qkvp.tile([128, NST, NH * D], rdt, name="v_sb")

                # ---------- Phase A(b): QKV + RoPE ----------
                phase_a(tc, io, b, xT, w_sb, qT_sb, kT_sb, v_sb, S, M, rdt,
                        ident_sb, xpool, ropetmp, packpool, vtpool,
                        load_w=(b == 0))

                # ---------- Phase B(b): causal attention ----------
                phase_b(tc, io, b, outT, ones_sb, tri_sb, qT_sb, kT_sb, v_sb,
                        S, rdt)


def phase_a(tc, io, b, xT, w_sb, qT_sb, kT_sb, v_sb, S, M, rdt, ident_sb,
            xpool, ropetmp, packpool, vtpool, load_w=False):
    nc = tc.nc
    MC = M // 128
    SLAB = 512
    NE = S // SLAB
    with (
        tc.tile_pool(name=f"psqk{b}", bufs=2, space="PSUM") as psqk,
        tc.tile_pool(name=f"psv{b}", bufs=1, space="PSUM") as psvp,
        tc.tile_pool(name=f"psT{b}", bufs=2, space="PSUM") as psT,
    ):
        pending_t = []

        def emit_transpose(vT_sb, h, e, st):
            tps = psT.tile([128, 128], rdt, tag="tps", name="tps")
            nc.tensor.transpose(
                tps[:], vT_sb[:, st * 128:(st + 1) * 128], ident_sb[:]
            )
            gst = e * (SLAB // 128) + st
            nc.vector.tensor_copy(v_sb[:, gst, h * D:(h + 1) * D], tps[:])

        xT_r = xT[b].rearrange("(mo p) s -> p mo s", p=128)
        for e in range(NE):
            sl = slice(e * SLAB, (e + 1) * SLAB)
            xe = xpool.tile([128, MC, SLAB], rdt, tag="xe", name="xe")
            if load_w and e == 0:
                # first slab: chunk the first few m so the m=0 matmuls
                # start immediately; bulk-load the rest (cheap issue)
                wq_r = io["wqT"].rearrange("(mo p) n -> p mo n", p=128)
                for m in range(4):
                    nc.sync.dma_start(
                        xe[:, m, :], xT[b][m * 128:(m + 1) * 128, sl]
                    )
                    nc.sync.dma_start(
                        w_sb["wqT"][:, m, :],
                        io["wqT"][m * 128:(m + 1) * 128, :],
                    )
                for g in range(4, MC, 4):
                    nc.sync.dma_start(
                        xe[:, g:g + 4, :], xT_r[:, g:g + 4, sl]
                    )
                    nc.sync.dma_start(
                        w_sb["wqT"][:, g:g + 4, :], wq_r[:, g:g + 4, :]
                    )
                for name in ("wkT", "wvT"):
                    nc.sync.dma_start(
                        w_sb[name][:],
                        io[name].rearrange("(mo p) n -> p mo n", p=128),
                    )
                nc.sync.dma_start(ident_sb[:], io["ident"][:])
            else:
                # prefetched slabs: one 3D-AP DMA (cheap issue)
                nc.sync.dma_start(xe[:], xT_r[:, :, sl])
            packC = packpool.tile([128, SLAB], F32, tag="packC",
                                  name="packC")
            packS = packpool.tile([128, SLAB], F32, tag="packS",
                                  name="packS")
            nc.sync.dma_start(packC[:], io["packC"][:, sl])
            nc.sync.dma_start(packS[:], io["packS"][:, sl])

            for name, dst in (("wqT", qT_sb), ("wkT", kT_sb)):
                ps = [
                    psqk.tile([128, SLAB], F32, tag=f"pqk{h}",
                              name=f"pqk{h}")
                    for h in range(NH)
                ]
                for m in range(MC):
                    for h in range(NH):
                        nc.tensor.matmul(
                            ps[h][:],
                            w_sb[name][:, m, h * D:(h + 1) * D],
                            xe[:, m, :],
                            start=(m == 0),
                            stop=(m == MC - 1),
                        )
                    # interleave a deferred v-transpose so its fused
                    # weight load hides under the wide Q/K streams
                    if name == "wqT" and m % 2 == 1 and pending_t:
                        emit_transpose(*pending_t.pop(0))
                for h in range(NH):
                    # quadrant-16 RoPE: out = ps*packC + shuffle16(ps*packS)
                    t1 = ropetmp.tile([128, SLAB], F32, tag="t1",
                                      name="t1")
                    t2 = ropetmp.tile([128, SLAB], F32, tag="t2",
                                      name="t2")
                    t2s = ropetmp.tile([128, SLAB], F32, tag="t2s",
                                       name="t2s")
                    nc.vector.tensor_mul(t1[:], ps[h][:], packC[:])
                    nc.vector.tensor_mul(t2[:], ps[h][:], packS[:])
                    nc.vector.stream_shuffle(t2s[:], t2[:], SWAP16)
                    nc.vector.tensor_add(dst[:, h, sl], t1[:], t2s[:])

            # v^T projection like q/k (wide moving dim), then
            # PE-transpose 128x128 blocks into the [s, n] layout
            psv = [
                psvp.tile([128, SLAB], F32, tag=f"pvt{h}",
                          name=f"pvt{h}")
                for h in range(NH)
            ]
            for m in range(MC):
                for h in range(NH):
                    nc.tensor.matmul(
                        psv[h][:],
                        w_sb["wvT"][:, m, h * D:(h + 1) * D],
                        xe[:, m, :],
                        start=(m == 0),
                        stop=(m == MC - 1),
                    )
            for h in range(NH):
                vT_sb = vtpool.tile([128, SLAB], rdt, tag="vT",
                                    name="vT_sb")
                nc.vector.tensor_copy(vT_sb[:], psv[h][:])
                for st in range(SLAB // 128):
                    pending_t.append((vT_sb, h, e, st))
        for args in pending_t:
            emit_transpose(*args)
        pending_t.clear()

def phase_b(tc, io, b, outT, ones_sb, tri_sb, qT_sb, kT_sb, v_sb, S, rdt):
    nc = tc.nc
    NQT = S // QT
    scale = 1.0 / math.sqrt(D)
    if b == 0:
        nc.sync.dma_start(tri_sb[:], io["tri"][:])
        nc.sync.dma_start(ones_sb[:], io["ones"][:])
    if True:
                with (
                    tc.tile_pool(name=f"expp{b}", bufs=4) as expp,
                    tc.tile_pool(name=f"outp{b}", bufs=2) as outp,
                    tc.tile_pool(name=f"psS{b}", bufs=4, space="PSUM") as psS,
                    tc.tile_pool(name=f"psO{b}", bufs=2, space="PSUM") as psO,
                    tc.tile_pool(name=f"psD{b}", bufs=2, space="PSUM") as psDen,
                ):
                    for h in range(NH):
                        u = b * NH + h
                        for qt in range(NQT):
                            nkt = (qt + 1) * (QT // 128)
                            out_ps = psO.tile([128, QT], F32, tag="out",
                                              name="out_ps")
                            den_ps = psDen.tile([128, QT], F32, tag="den",
                                                name="den_ps")

                            pend = []

                            def tail(expS, kt, rs, nkt=nkt, out_ps=out_ps,
                                     den_ps=den_ps, h=h):
                                nc.tensor.matmul(
                                    den_ps[:, rs:],
                                    ones_sb[:],
                                    expS[:, rs:],
                                    start=(kt == 0),
                                    stop=(kt == nkt - 1),
                                )
                                nc.tensor.matmul(
                                    out_ps[:, rs:],
                                    v_sb[:, kt, h * D:(h + 1) * D],
                                    expS[:, rs:],
                                    start=(kt == 0),
                                    stop=(kt == nkt - 1),
                                )

                            for kt in range(nkt):
                                j = kt - (nkt - 4)
                                # fp32r matmuls need a moving dim >= 256 for
                                # full rate, so the last diagonal tile (j=3)
                                # widens to 256 and zeroes the masked strip
                                rs = 128 * j if j > 0 else 0
                                if j == 3:
                                    rs = 256
                                s_ps = psS.tile([128, QT], F32, tag="s",
                                                name="s_ps")
                                nc.tensor.matmul(
                                    s_ps[:, rs:],
                                    kT_sb[:, h, kt * 128:(kt + 1) * 128],
                                    qT_sb[:, h, qt * QT + rs:(qt + 1) * QT],
                                    start=True,
                                    stop=True,
                                )
                                expS = expp.tile([128, QT], rdt, tag="exp",
                                                 name="expS")
                                nc.scalar.activation(
                                    expS[:, rs:], s_ps[:, rs:], AF.Exp,
                                    scale=scale,
                                )
                                if j == 3:
                                    nc.vector.tensor_scalar_mul(
                                        expS[:, 256:384], expS[:, 256:384], 0.0
                                    )
                                if j >= 0:
                                    nc.vector.tensor_mul(
                                        expS[:, 128 * j:128 * (j + 1)],
                                        expS[:, 128 * j:128 * (j + 1)],
                                        tri_sb[:],
                                    )
                                pend.append((expS, kt, rs))
                                if len(pend) > 3:
                                    tail(*pend.pop(0))
                            while pend:
                                tail(*pend.pop(0))

                            recip = outp.tile([128, QT], F32, tag="recip",
                                              name="recip")
                            nc.vector.reciprocal_approx_fast(recip[:],
                                                             den_ps[:])
                            o_sb = outp.tile([128, QT], F32, tag="o",
                                             name="o_sb")
                            nc.vector.tensor_mul(o_sb[:], out_ps[:], recip[:])
                            nc.sync.dma_start(
                                outT[u, :, qt * QT:(qt + 1) * QT], o_sb[:]
                            )

_NC_CACHE = {}


def _get_nc():
    if "nc" not in _NC_CACHE:
        nc = bacc.Bacc(
            "TRN2", target_bir_lowering=False, debug=False, num_devices=NCORES
        )
        io = {}
        for name, shape, dt_ in (
            ("xT0", [M, S], F32R),
            ("xT1", [M, S], F32R),
            ("wqT", [M, NH * D], F32R),
            ("wkT", [M, NH * D], F32R),
            ("wvT", [M, NH * D], F32R),
            ("packC", [128, S], F32),
            ("packS", [128, S], F32),
            ("tri", [128, 128], F32R),
            ("ones", [128, 128], F32R),
            ("ident", [128, 128], F32R),
        ):
            io[name] = nc.dram_tensor(name, shape, dt_, kind="ExternalInput").ap()
        io["outT"] = nc.dram_tensor(
            "outT", [NB * NH, 128, S], F32, kind="ExternalOutput"
        ).ap()
        with tile.TileContext(nc) as tc:
            build_attention(tc, io, S, M)
        nc.compile()
        _NC_CACHE["nc"] = nc
    return _NC_CACHE["nc"]


def kernel(x, Wq, Wk, Wv):
    x = np.asarray(x, dtype=np.float32)
    Wq = np.asarray(Wq, dtype=np.float32)
    Wk = np.asarray(Wk, dtype=np.float32)
    Wv = np.asarray(Wv, dtype=np.float32)

    nc = _get_nc()
    in_maps = [prep_core_inputs(x, Wq, Wk, Wv, c, S, M) for c in range(NCORES)]
    res = bass_utils.run_bass_kernel_spmd(nc, in_maps, core_ids=list(range(NCORES)))

    out = np.empty((NB, S, M), dtype=np.float32)
    for c in range(NCORES):
        outT = res.results[c]["outT"]
        for u in range(NB * NH):
            b, hl = u // NH, u % NH
            col = c * NH * D + hl * D
            out[b, :, col:col + D] = outT[u].T
    return out



# revision 8
# speedup vs baseline: 1.1281x; 1.1281x over previous
"""Trainium2 Bass kernel for causal multi-head attention with RoPE.

nn_CausalAttention: x [2, 2048, 2048], Wq/Wk/Wv [2048, 2048] (y = x @ W.T),
16 heads of dim 128, RoPE, causal fp32 softmax.

Sharding (tensor-parallel heads): each of the 8 NeuronCores owns 2 heads (a
256-wide slice of the QKV output dim) for both batch elements. The full
output is assembled on host by concatenating per-core feature slices.

v3 highlights (vs the fp32r v2 baseline):
  * all matmul operands are fp16 (fp32 PSUM accumulation). Numerics sim'd at
    rel err ~6e-4 vs the fp32 reference (gate is 2e-2). Halves DMA/SBUF
    traffic and enables the DVE 2x/4x 16-bit modes.
  * V is projected directly into [seq, dim] layout (x-chunk stationary,
    Wv^T moving) - no PE transposes, no double PSUM->SBUF copies.
  * softmax denominator is accumulated on the DVE (fp16 elementwise adds of
    the exp tiles) with a single ones-matmul per 512-query tile, removing
    the per-key-tile ones-matmul stream (~30us of PE time).
  * exp computes e^(s*scale - 4): the constant bias cancels in num/den and
    keeps fp16 exp values far from overflow.
  * diagonal score tiles sub-range to exact 128-col multiples (fp16 has no
    >=256 moving-dim constraint), killing the widen+zero hack.
  * PSUM->SBUF fp16 evictions ride the DVE; the scalar engine does exp only.
"""

import math

import numpy as np

import concourse.bacc as bacc
import concourse.bass as bass
import concourse.mybir as mybir
import concourse.tile as tile
from concourse import bass_utils

F32 = mybir.dt.float32
F16 = mybir.dt.float16
AF = mybir.ActivationFunctionType

S = 2048
M = 2048
NCORES = 8

D = 128          # head dim
NH = 2           # heads per core
NB = 2           # batches
SLAB = 512       # phase-A sequence slab
QT = 512         # phase-B query tile
EXP_BIAS = -4.0  # exp(s*scale + EXP_BIAS); cancels in softmax ratio


def _rope_perm(n):
    """Row permutation for the quadrant-16 RoPE layout.

    New row p (within a 128-row head block): quadrant qd = p//32, r = p%32.
    r < 16  -> even element of pair i = 16*qd + r      (old row 2i)
    r >= 16 -> odd  element of pair i = 16*qd + (r-16) (old row 2i+1)
    Pair elements are 16 partitions apart inside one 32-partition quadrant,
    so the RoPE combine is a stream_shuffle with a 16-rotation mask.
    """
    perm = []
    for hb in range(n // D):
        base = hb * D
        for qd in range(4):
            perm += [base + 2 * (16 * qd + r) for r in range(16)]
            perm += [base + 2 * (16 * qd + r) + 1 for r in range(16)]
    return np.array(perm)


SWAP16 = [(i + 16) % 32 for i in range(32)]

_HOST_CACHE = {}


def _host_shared(x, Wq, Wk, Wv):
    """fp16 conversions shared by all 8 cores (computed once per input set)."""
    key = (id(x), id(Wq))
    if key in _HOST_CACHE:
        return _HOST_CACHE[key]
    theta = np.exp(
        -np.float32(np.log(10000.0))
        * (np.arange(0, D, 2, dtype=np.float32) / np.float32(D))
    ).astype(np.float32)
    pos = np.arange(S, dtype=np.float32)
    freqs = theta[:, None] * pos[None, :]  # [64, S]
    cos_t, sin_t = np.cos(freqs), np.sin(freqs)
    p = np.arange(128)
    i_of_p = 16 * (p // 32) + (p % 16)
    is_odd = (p % 32) >= 16
    packC = cos_t[i_of_p].astype(np.float16)                     # [128, S]
    packS = np.where(
        is_odd[:, None], -sin_t[i_of_p], sin_t[i_of_p]
    ).astype(np.float16)

    kk, qq = np.meshgrid(np.arange(128), np.arange(128), indexing="ij")
    tri = (kk <= qq).astype(np.float16)

    shared = {
        "xT0": np.ascontiguousarray(x[0].T).astype(np.float16),
        "xT1": np.ascontiguousarray(x[1].T).astype(np.float16),
        "packC": packC,
        "packS": packS,
        "tri": tri,
        "ones": np.ones((128, 128), dtype=np.float16),
        "bias4": np.full((128, 1), EXP_BIAS, dtype=np.float32),
    }
    _HOST_CACHE.clear()
    _HOST_CACHE[key] = shared
    return shared


def prep_core_inputs(x, Wq, Wk, Wv, core, S, M):
    """Host-side shard prep for one core. x [2,S,M], W* [M', M] where
    rows [core*256, core*256+256) of W* are this core's heads."""
    shared = _host_shared(x, Wq, Wk, Wv)
    nsl = slice(core * NH * D, (core + 1) * NH * D)
    perm = _rope_perm(NH * D)
    wq = Wq[nsl][perm]
    wk = Wk[nsl][perm]
    wv = Wv[nsl]
    io = dict(shared)
    io["wqT"] = np.ascontiguousarray(wq.T).astype(np.float16)
    io["wkT"] = np.ascontiguousarray(wk.T).astype(np.float16)
    io["wvT"] = np.ascontiguousarray(wv.T).astype(np.float16)
    return io


def build_attention(tc: tile.TileContext, io: dict, S: int, M: int):
    nc = tc.nc
    xT = [io["xT0"], io["xT1"]]
    outT = io["outT"]

    with (
        tc.tile_pool(name="wpool", bufs=1) as wpool,
        tc.tile_pool(name="constpool", bufs=1) as constpool,
        tc.tile_pool(name="xp", bufs=2) as xpool,
        tc.tile_pool(name="rope", bufs=2) as ropetmp,
        tc.tile_pool(name="pack", bufs=1) as packpool,
    ):
        MC = M // 128
        w_sb = {}
        for name in ("wqT", "wkT", "wvT"):
            w_sb[name] = wpool.tile([128, MC, NH * D], F16, tag=name, name=name)
        tri_sb = constpool.tile([128, 128], F16)
        ones_sb = constpool.tile([128, 128], F16)
        bias_sb = constpool.tile([128, 1], F32)

        for b in range(NB):
            with tc.tile_pool(name=f"qkv{b}", bufs=1) as qkvp:
                qT_sb = qkvp.tile([128, NH, S], F16, name="qT_sb")
                kT_sb = qkvp.tile([128, NH, S], F16, name="kT_sb")
                v_sb = qkvp.tile([128, S // 128, NH * D], F16, name="v_sb")

                phase_a(tc, io, b, xT, w_sb, qT_sb, kT_sb, v_sb, S, M,
                        xpool, ropetmp, packpool, load_w=(b == 0))
                phase_b(tc, io, b, outT, ones_sb, tri_sb, bias_sb,
                        qT_sb, kT_sb, v_sb, S)


def phase_a(tc, io, b, xT, w_sb, qT_sb, kT_sb, v_sb, S, M,
            xpool, ropetmp, packpool, load_w=False):
    nc = tc.nc
    MC = M // 128
    NE = S // SLAB
    with (
        tc.tile_pool(name=f"psqk{b}", bufs=2, space="PSUM") as psqk,
        tc.tile_pool(name=f"psv{b}", bufs=2, space="PSUM") as psvp,
    ):
        xT_r = xT[b].rearrange("(mo p) s -> p mo s", p=128)
        for e in range(NE):
            sl = slice(e * SLAB, (e + 1) * SLAB)
            xe = xpool.tile([128, MC, SLAB], F16, tag="xe", name="xe")
            if load_w and e == 0:
                # first slab: chunk the first few m so the m=0 matmuls
                # start immediately; bulk-load the rest (cheap issue)
                wq_r = io["wqT"].rearrange("(mo p) n -> p mo n", p=128)
                for m in range(4):
                    nc.sync.dma_start(
                        xe[:, m, :], xT[b][m * 128:(m + 1) * 128, sl]
                    )
                    nc.sync.dma_start(
                        w_sb["wqT"][:, m, :],
                        io["wqT"][m * 128:(m + 1) * 128, :],
                    )
                for g in range(4, MC, 4):
                    nc.sync.dma_start(
                        xe[:, g:g + 4, :], xT_r[:, g:g + 4, sl]
                    )
                    nc.sync.dma_start(
                        w_sb["wqT"][:, g:g + 4, :], wq_r[:, g:g + 4, :]
                    )
                for name in ("wkT", "wvT"):
                    nc.sync.dma_start(
                        w_sb[name][:],
                        io[name].rearrange("(mo p) n -> p mo n", p=128),
                    )
            else:
                nc.sync.dma_start(xe[:], xT_r[:, :, sl])
            packC = packpool.tile([128, SLAB], F16, tag="packC",
                                  name="packC")
            packS = packpool.tile([128, SLAB], F16, tag="packS",
                                  name="packS")
            nc.sync.dma_start(packC[:], io["packC"][:, sl])
            nc.sync.dma_start(packS[:], io["packS"][:, sl])

            # ---- Q/K projections: W-chunk stationary, x moving (512) ----
            for name, dst in (("wqT", qT_sb), ("wkT", kT_sb)):
                ps = [
                    psqk.tile([128, SLAB], F32, tag=f"pqk{h}",
                              name=f"pqk{h}")
                    for h in range(NH)
                ]
                for m in range(MC):
                    for h in range(NH):
                        nc.tensor.matmul(
                            ps[h][:],
                            w_sb[name][:, m, h * D:(h + 1) * D],
                            xe[:, m, :],
                            start=(m == 0),
                            stop=(m == MC - 1),
                        )
                for h in range(NH):
                    # quadrant-16 RoPE on DVE: eviction copy to fp16, then
                    # 16-bit 4x-mode mul/mul/shuffle/add
                    p16 = ropetmp.tile([128, SLAB], F16, tag="p16",
                                       name="p16")
                    t1 = ropetmp.tile([128, SLAB], F16, tag="t1", name="t1")
                    t2 = ropetmp.tile([128, SLAB], F16, tag="t2", name="t2")
                    t2s = ropetmp.tile([128, SLAB], F16, tag="t2s",
                                       name="t2s")
                    nc.vector.tensor_copy(p16[:], ps[h][:])
                    nc.vector.tensor_mul(t1[:], p16[:], packC[:])
                    nc.vector.tensor_mul(t2[:], p16[:], packS[:])
                    nc.vector.stream_shuffle(t2s[:], t2[:], SWAP16)
                    nc.vector.tensor_add(dst[:, h, sl], t1[:], t2s[:])

            # ---- V projection directly in [seq, n] layout:
            #      x-chunk stationary, Wv^T moving (256) ----
            for st in range(SLAB // 128):
                pv = psvp.tile([128, NH * D], F32, tag="pv", name="pv")
                for m in range(MC):
                    nc.tensor.matmul(
                        pv[:],
                        xe[:, m, st * 128:(st + 1) * 128],
                        w_sb["wvT"][:, m, :],
                        start=(m == 0),
                        stop=(m == MC - 1),
                    )
                gst = e * (SLAB // 128) + st
                nc.vector.tensor_copy(v_sb[:, gst, :], pv[:])


def phase_b(tc, io, b, outT, ones_sb, tri_sb, bias_sb, qT_sb, kT_sb, v_sb, S):
    nc = tc.nc
    NQT = S // QT
    scale = 1.0 / math.sqrt(D)
    if b == 0:
        nc.sync.dma_start(tri_sb[:], io["tri"][:])
        nc.sync.dma_start(ones_sb[:], io["ones"][:])
        nc.sync.dma_start(bias_sb[:], io["bias4"][:])
    with (
        tc.tile_pool(name=f"expp{b}", bufs=4) as expp,
        tc.tile_pool(name=f"denp{b}", bufs=2) as denp,
        tc.tile_pool(name=f"outp{b}", bufs=2) as outp,
        tc.tile_pool(name=f"psS{b}", bufs=3, space="PSUM") as psS,
        tc.tile_pool(name=f"psO{b}", bufs=2, space="PSUM") as psO,
        tc.tile_pool(name=f"psD{b}", bufs=1, space="PSUM") as psDen,
    ):
        for h in range(NH):
            u = b * NH + h
            for qt in range(NQT):
                nkt = (qt + 1) * (QT // 128)
                out_ps = psO.tile([128, QT], F32, tag="out", name="out_ps")
                den_acc = denp.tile([128, QT], F16, tag="den",
                                    name="den_acc")

                pend = []

                def av_tail(expS, kt, rs, nkt=nkt, out_ps=out_ps, h=h):
                    nc.tensor.matmul(
                        out_ps[:, rs:],
                        v_sb[:, kt, h * D:(h + 1) * D],
                        expS[:, rs:],
                        start=(kt == 0),
                        stop=(kt == nkt - 1),
                    )

                for kt in range(nkt):
                    j = kt - (nkt - 4)
                    rs = 128 * j if j > 0 else 0
                    s_ps = psS.tile([128, QT], F32, tag="s", name="s_ps")
                    nc.tensor.matmul(
                        s_ps[:, rs:],
                        kT_sb[:, h, kt * 128:(kt + 1) * 128],
                        qT_sb[:, h, qt * QT + rs:(qt + 1) * QT],
                        start=True,
                        stop=True,
                    )
                    expS = expp.tile([128, QT], F16, tag="exp", name="expS")
                    nc.scalar.activation(
                        expS[:, rs:], s_ps[:, rs:], AF.Exp,
                        bias=bias_sb[:], scale=scale,
                    )
                    if j >= 0:
                        nc.vector.tensor_mul(
                            expS[:, 128 * j:128 * (j + 1)],
                            expS[:, 128 * j:128 * (j + 1)],
                            tri_sb[:],
                        )
                    # fp16 DVE accumulation of the softmax denominator
                    if kt == 0:
                        nc.vector.tensor_copy(den_acc[:], expS[:])
                    else:
                        nc.vector.tensor_add(
                            den_acc[:, rs:], den_acc[:, rs:], expS[:, rs:]
                        )
                    pend.append((expS, kt, rs))
                    if len(pend) > 3:
                        av_tail(*pend.pop(0))
                while pend:
                    av_tail(*pend.pop(0))

                den_ps = psDen.tile([128, QT], F32, tag="denp",
                                    name="den_ps")
                nc.tensor.matmul(
                    den_ps[:], ones_sb[:], den_acc[:], start=True, stop=True
                )
                recip = outp.tile([128, QT], F32, tag="recip", name="recip")
                nc.vector.reciprocal_approx_fast(recip[:], den_ps[:])
                o_sb = outp.tile([128, QT], F32, tag="o", name="o_sb")
                nc.vector.tensor_mul(o_sb[:], out_ps[:], recip[:])
                nc.sync.dma_start(
                    outT[u, :, qt * QT:(qt + 1) * QT], o_sb[:]
                )


_NC_CACHE = {}


def _get_nc():
    if "nc" not in _NC_CACHE:
        nc = bacc.Bacc(
            "TRN2", target_bir_lowering=False, debug=False, num_devices=NCORES
        )
        io = {}
        for name, shape, dt_ in (
            ("xT0", [M, S], F16),
            ("xT1", [M, S], F16),
            ("wqT", [M, NH * D], F16),
            ("wkT", [M, NH * D], F16),
            ("wvT", [M, NH * D], F16),
            ("packC", [128, S], F16),
            ("packS", [128, S], F16),
            ("tri", [128, 128], F16),
            ("ones", [128, 128], F16),
            ("bias4", [128, 1], F32),
        ):
            io[name] = nc.dram_tensor(name, shape, dt_, kind="ExternalInput").ap()
        io["outT"] = nc.dram_tensor(
            "outT", [NB * NH, 128, S], F32, kind="ExternalOutput"
        ).ap()
        with tile.TileContext(nc) as tc:
            build_attention(tc, io, S, M)
        nc.compile()
        _NC_CACHE["nc"] = nc
    return _NC_CACHE["nc"]


def kernel(x, Wq, Wk, Wv):
    x = np.asarray(x, dtype=np.float32)
    Wq = np.asarray(Wq, dtype=np.float32)
    Wk = np.asarray(Wk, dtype=np.float32)
    Wv = np.asarray(Wv, dtype=np.float32)

    nc = _get_nc()
    in_maps = [prep_core_inputs(x, Wq, Wk, Wv, c, S, M) for c in range(NCORES)]
    res = bass_utils.run_bass_kernel_spmd(nc, in_maps, core_ids=list(range(NCORES)))

    out = np.empty((NB, S, M), dtype=np.float32)
    for c in range(NCORES):
        outT = res.results[c]["outT"]
        for u in range(NB * NH):
            b, hl = u // NH, u % NH
            col = c * NH * D + hl * D
            out[b, :, col:col + D] = outT[u].T
    return out


# revision 9
# speedup vs baseline: 1.2644x; 1.1208x over previous
"""Trainium2 Bass kernel for causal multi-head attention with RoPE.

nn_CausalAttention: x [2, 2048, 2048], Wq/Wk/Wv [2048, 2048] (y = x @ W.T),
16 heads of dim 128, RoPE, causal fp32 softmax.

Sharding (tensor-parallel heads): each of the 8 NeuronCores owns 2 heads (a
256-wide slice of the QKV output dim) for both batch elements. The full
output is assembled on host by concatenating per-core feature slices.

v4: fully software-pipelined schedule. All matmul operands are fp16 (fp32
PSUM accumulation, numerics ~6e-4 vs the fp32 reference). The kernel is a
single stream of projection slabs (Q/K proj + RoPE + direct-[seq,dim] V
proj per 512-position slab) with causal-attention "tile groups"
(score-matmul -> exp -> mask -> DVE denominator accumulate -> attn@V) fed
between projection matmuls as soon as their slab dependencies are emitted.
This spreads the scalar-engine exp work (the phase-B bottleneck) across the
whole timeline instead of bunching it after each batch's projections, so
the PE almost never waits on exp. The softmax denominator is accumulated on
the DVE in fp16 and reduced across key lanes with a single ones-matmul per
512-query tile. exp computes e^(s*scale - 4) (bias cancels in the softmax
ratio; keeps fp16 exp well inside range).

PSUM budget (8 banks): Q/K accumulators 2 (bufs=1, evicted to fp16 by DVE
before reuse), V pair-accumulator 1 (two 256-col s-tiles share one bank and
one accumulation group), scores 2, attention output 2, denominator 1.
"""

import math
from collections import deque

import numpy as np

import concourse.bacc as bacc
import concourse.bass as bass
import concourse.mybir as mybir
import concourse.tile as tile
from concourse import bass_utils

F32 = mybir.dt.float32
F16 = mybir.dt.float16
AF = mybir.ActivationFunctionType

S = 2048
M = 2048
NCORES = 8

D = 128          # head dim
NH = 2           # heads per core
NB = 2           # batches
SLAB = 512       # phase-A sequence slab == phase-B query tile
QT = 512
NE = S // SLAB
EXP_BIAS = -4.0  # exp(s*scale + EXP_BIAS); cancels in softmax ratio


def _rope_perm(n):
    """Row permutation for the quadrant-16 RoPE layout.

    New row p (within a 128-row head block): quadrant qd = p//32, r = p%32.
    r < 16  -> even element of pair i = 16*qd + r      (old row 2i)
    r >= 16 -> odd  element of pair i = 16*qd + (r-16) (old row 2i+1)
    Pair elements are 16 partitions apart inside one 32-partition quadrant,
    so the RoPE combine is a stream_shuffle with a 16-rotation mask.
    """
    perm = []
    for hb in range(n // D):
        base = hb * D
        for qd in range(4):
            perm += [base + 2 * (16 * qd + r) for r in range(16)]
            perm += [base + 2 * (16 * qd + r) + 1 for r in range(16)]
    return np.array(perm)


SWAP16 = [(i + 16) % 32 for i in range(32)]

_HOST_CACHE = {}


def _host_shared(x):
    """fp16 conversions shared by all 8 cores (computed once per input set)."""
    key = id(x)
    if key in _HOST_CACHE:
        return _HOST_CACHE[key]
    theta = np.exp(
        -np.float32(np.log(10000.0))
        * (np.arange(0, D, 2, dtype=np.float32) / np.float32(D))
    ).astype(np.float32)
    pos = np.arange(S, dtype=np.float32)
    freqs = theta[:, None] * pos[None, :]  # [64, S]
    cos_t, sin_t = np.cos(freqs), np.sin(freqs)
    p = np.arange(128)
    i_of_p = 16 * (p // 32) + (p % 16)
    is_odd = (p % 32) >= 16
    packC = cos_t[i_of_p].astype(np.float16)                     # [128, S]
    packS = np.where(
        is_odd[:, None], -sin_t[i_of_p], sin_t[i_of_p]
    ).astype(np.float16)

    kk, qq = np.meshgrid(np.arange(128), np.arange(128), indexing="ij")
    tri = (kk <= qq).astype(np.float16)

    shared = {
        "xT0": np.ascontiguousarray(x[0].T).astype(np.float16),
        "xT1": np.ascontiguousarray(x[1].T).astype(np.float16),
        "packC": packC,
        "packS": packS,
        "tri": tri,
        "ones": np.ones((128, 128), dtype=np.float16),
        "bias4": np.full((128, 1), EXP_BIAS, dtype=np.float32),
    }
    _HOST_CACHE.clear()
    _HOST_CACHE[key] = shared
    return shared


def prep_core_inputs(x, Wq, Wk, Wv, core, S, M):
    """Host-side shard prep for one core. x [2,S,M], W* [M', M] where
    rows [core*256, core*256+256) of W* are this core's heads."""
    shared = _host_shared(x)
    nsl = slice(core * NH * D, (core + 1) * NH * D)
    perm = _rope_perm(NH * D)
    io = dict(shared)
    io["wqT"] = np.ascontiguousarray(Wq[nsl][perm].T).astype(np.float16)
    io["wkT"] = np.ascontiguousarray(Wk[nsl][perm].T).astype(np.float16)
    io["wvT"] = np.ascontiguousarray(Wv[nsl].T).astype(np.float16)
    return io


class Feeder:
    """Queue of deferred attention tile-group emitters, drained between
    projection matmul units so exp latency hides under PE work."""

    def __init__(self, units_per_slab):
        self.q = deque()
        self.units_per_slab = units_per_slab
        self.cnt = 0
        self.k = units_per_slab

    def enqueue(self, fns):
        self.q.extend(fns)
        self.k = max(1, self.units_per_slab // max(1, len(self.q)))

    def tick(self):
        self.cnt += 1
        if self.q and self.cnt % self.k == 0:
            self.q.popleft()()

    def drain(self):
        while self.q:
            self.q.popleft()()


def build_attention(tc: tile.TileContext, io: dict, S: int, M: int):
    nc = tc.nc
    MC = M // 128
    scale = 1.0 / math.sqrt(D)
    xT = [io["xT0"], io["xT1"]]
    outT = io["outT"]

    with (
        tc.tile_pool(name="wpool", bufs=1) as wpool,
        tc.tile_pool(name="constpool", bufs=1) as constpool,
        tc.tile_pool(name="xp", bufs=2) as xpool,
        tc.tile_pool(name="rope", bufs=2) as ropetmp,
        tc.tile_pool(name="pack", bufs=1) as packpool,
        tc.tile_pool(name="qkv", bufs=2) as qkvp,
        tc.tile_pool(name="expp", bufs=8) as expp,
        tc.tile_pool(name="denp", bufs=2) as denp,
        tc.tile_pool(name="outp", bufs=2) as outp,
        tc.tile_pool(name="psqk", bufs=1, space="PSUM") as psqk,
        tc.tile_pool(name="psv", bufs=1, space="PSUM") as psvp,
        tc.tile_pool(name="psS", bufs=2, space="PSUM") as psS,
        tc.tile_pool(name="psO", bufs=2, space="PSUM") as psO,
        tc.tile_pool(name="psD", bufs=1, space="PSUM") as psDen,
    ):
        w_sb = {}
        for name in ("wqT", "wkT", "wvT"):
            w_sb[name] = wpool.tile([128, MC, NH * D], F16, tag=name, name=name)
        tri_sb = constpool.tile([128, 128], F16)
        ones_sb = constpool.tile([128, 128], F16)
        bias_sb = constpool.tile([128, 1], F32)

        # projection PE units per slab: 32 QK pairs + 16 V pairs
        feeder = Feeder(units_per_slab=48)

        # ---------------- attention tile-group machinery ----------------
        def make_qt_state(b, qT_sb, kT_sb, v_sb):
            """Per-(b,qt) state: for each head an out accumulator, den
            accumulator and av pend queue, created lazily at kt=0."""
            return {"b": b, "qT": qT_sb, "kT": kT_sb, "v": v_sb,
                    "out_ps": {}, "den": {}, "pend": {h: [] for h in range(NH)}}

        def make_group(st, h, qt, kt, nkt):
            def emit():
                qT_sb, kT_sb, v_sb = st["qT"], st["kT"], st["v"]
                j = kt - (nkt - 4)
                rs = 128 * j if j > 0 else 0
                if kt == 0:
                    st["out_ps"][h] = psO.tile([128, QT], F32, tag="out",
                                               name="out_ps")
                    st["den"][h] = denp.tile([128, QT], F16, tag=f"den{h}",
                                             name="den_acc")
                out_ps, den_acc = st["out_ps"][h], st["den"][h]
                pend = st["pend"][h]

                s_ps = psS.tile([128, QT], F32, tag="s", name="s_ps")
                nc.tensor.matmul(
                    s_ps[:, rs:],
                    kT_sb[:, h, kt * 128:(kt + 1) * 128],
                    qT_sb[:, h, qt * QT + rs:(qt + 1) * QT],
                    start=True,
                    stop=True,
                )
                expS = expp.tile([128, QT], F16, tag=f"exp{h}", name="expS")
                nc.scalar.activation(
                    expS[:, rs:], s_ps[:, rs:], AF.Exp,
                    bias=bias_sb[:], scale=scale,
                )
                if j >= 0:
                    nc.vector.tensor_mul(
                        expS[:, 128 * j:128 * (j + 1)],
                        expS[:, 128 * j:128 * (j + 1)],
                        tri_sb[:],
                    )
                if kt == 0:
                    nc.vector.tensor_copy(den_acc[:], expS[:])
                else:
                    nc.vector.tensor_add(
                        den_acc[:, rs:], den_acc[:, rs:], expS[:, rs:]
                    )
                pend.append((expS, kt, rs))
                if len(pend) > 3:
                    av_tail(st, h, nkt, *pend.pop(0))
                if kt == nkt - 1:
                    while pend:
                        av_tail(st, h, nkt, *pend.pop(0))
                    finalize(st, h, qt)
            return emit

        def av_tail(st, h, nkt, expS, kt, rs):
            nc.tensor.matmul(
                st["out_ps"][h][:, rs:],
                st["v"][:, kt, h * D:(h + 1) * D],
                expS[:, rs:],
                start=(kt == 0),
                stop=(kt == nkt - 1),
            )

        def finalize(st, h, qt):
            u = st["b"] * NH + h
            den_ps = psDen.tile([128, QT], F32, tag="denp", name="den_ps")
            nc.tensor.matmul(
                den_ps[:], ones_sb[:], st["den"][h][:], start=True, stop=True
            )
            recip = outp.tile([128, QT], F32, tag="recip", name="recip")
            nc.vector.reciprocal_approx_fast(recip[:], den_ps[:])
            o_sb = outp.tile([128, QT], F32, tag="o", name="o_sb")
            nc.vector.tensor_mul(o_sb[:], st["out_ps"][h][:], recip[:])
            nc.sync.dma_start(
                outT[u, :, qt * QT:(qt + 1) * QT], o_sb[:]
            )

        # ---------------- projection slab emission ----------------
        for b in range(NB):
            qT_sb = qkvp.tile([128, NH, S], F16, tag="qT", name="qT_sb")
            kT_sb = qkvp.tile([128, NH, S], F16, tag="kT", name="kT_sb")
            v_sb = qkvp.tile([128, S // 128, NH * D], F16, tag="v",
                             name="v_sb")
            xT_r = xT[b].rearrange("(mo p) s -> p mo s", p=128)

            for e in range(NE):
                sl = slice(e * SLAB, (e + 1) * SLAB)
                xe = xpool.tile([128, MC, SLAB], F16, tag="xe", name="xe")
                if b == 0 and e == 0:
                    # first slab: chunk the first few m so the m=0 matmuls
                    # start immediately; bulk-load the rest (cheap issue)
                    wq_r = io["wqT"].rearrange("(mo p) n -> p mo n", p=128)
                    for m in range(4):
                        nc.sync.dma_start(
                            xe[:, m, :], xT[b][m * 128:(m + 1) * 128, sl]
                        )
                        nc.sync.dma_start(
                            w_sb["wqT"][:, m, :],
                            io["wqT"][m * 128:(m + 1) * 128, :],
                        )
                    for g in range(4, MC, 4):
                        nc.sync.dma_start(
                            xe[:, g:g + 4, :], xT_r[:, g:g + 4, sl]
                        )
                        nc.sync.dma_start(
                            w_sb["wqT"][:, g:g + 4, :], wq_r[:, g:g + 4, :]
                        )
                    for name in ("wkT", "wvT"):
                        nc.sync.dma_start(
                            w_sb[name][:],
                            io[name].rearrange("(mo p) n -> p mo n", p=128),
                        )
                    nc.sync.dma_start(tri_sb[:], io["tri"][:])
                    nc.sync.dma_start(ones_sb[:], io["ones"][:])
                    nc.sync.dma_start(bias_sb[:], io["bias4"][:])
                else:
                    nc.sync.dma_start(xe[:], xT_r[:, :, sl])
                packC = packpool.tile([128, SLAB], F16, tag="packC",
                                      name="packC")
                packS = packpool.tile([128, SLAB], F16, tag="packS",
                                      name="packS")
                nc.sync.dma_start(packC[:], io["packC"][:, sl])
                nc.sync.dma_start(packS[:], io["packS"][:, sl])

                st = make_qt_state(b, qT_sb, kT_sb, v_sb)
                nkt = (e + 1) * (QT // 128)

                def rope(ps, dst, h):
                    p16 = ropetmp.tile([128, SLAB], F16, tag="p16",
                                       name="p16")
                    t1 = ropetmp.tile([128, SLAB], F16, tag="t1", name="t1")
                    t2 = ropetmp.tile([128, SLAB], F16, tag="t2", name="t2")
                    t2s = ropetmp.tile([128, SLAB], F16, tag="t2s",
                                       name="t2s")
                    nc.vector.tensor_copy(p16[:], ps[:])
                    nc.vector.tensor_mul(t1[:], p16[:], packC[:])
                    nc.vector.tensor_mul(t2[:], p16[:], packS[:])
                    nc.vector.stream_shuffle(t2s[:], t2[:], SWAP16)
                    nc.vector.tensor_add(dst[:, h, sl], t1[:], t2s[:])

                # --- Q projection (both heads), RoPE ---
                ps = {h: psqk.tile([128, SLAB], F32, tag=f"pqk{h}",
                                   name=f"pqk{h}") for h in range(NH)}
                for m in range(MC):
                    for h in range(NH):
                        nc.tensor.matmul(
                            ps[h][:],
                            w_sb["wqT"][:, m, h * D:(h + 1) * D],
                            xe[:, m, :],
                            start=(m == 0),
                            stop=(m == MC - 1),
                        )
                    feeder.tick()
                for h in range(NH):
                    rope(ps[h], qT_sb, h)

                # Q(slab e) ready -> off-diagonal groups for qt=e (need
                # keys/values only from earlier slabs)
                groups = []
                for kt in range(4 * e):
                    for h in range(NH):
                        groups.append(make_group(st, h, e, kt, nkt))
                feeder.enqueue(groups)

                # --- V projection for s-tiles 0,1 (paired in one bank) ---
                def vpair(st0):
                    pv = psvp.tile([128, 2, NH * D], F32, tag="pv",
                                   name="pv")
                    for m in range(MC):
                        for i in range(2):
                            nc.tensor.matmul(
                                pv[:, i, :],
                                xe[:, m,
                                   (st0 + i) * 128:(st0 + i + 1) * 128],
                                w_sb["wvT"][:, m, :],
                                start=(m == 0 and i == 0),
                                stop=(m == MC - 1 and i == 1),
                                skip_group_check=True,
                            )
                        feeder.tick()
                    gst = e * (SLAB // 128) + st0
                    nc.vector.tensor_copy(
                        v_sb[:, gst:gst + 2, :], pv[:]
                    )

                vpair(0)

                # --- K projection (both heads), RoPE ---
                for m in range(MC):
                    for h in range(NH):
                        nc.tensor.matmul(
                            ps[h][:],
                            w_sb["wkT"][:, m, h * D:(h + 1) * D],
                            xe[:, m, :],
                            start=(m == 0),
                            stop=(m == MC - 1),
                        )
                    feeder.tick()
                for h in range(NH):
                    rope(ps[h], kT_sb, h)

                vpair(2)

                # K/V(slab e) ready -> diagonal groups for qt=e
                groups = []
                for kt in range(4 * e, nkt):
                    for h in range(NH):
                        groups.append(make_group(st, h, e, kt, nkt))
                feeder.enqueue(groups)

        feeder.drain()


_NC_CACHE = {}


def _get_nc():
    if "nc" not in _NC_CACHE:
        nc = bacc.Bacc(
            "TRN2", target_bir_lowering=False, debug=False, num_devices=NCORES
        )
        io = {}
        for name, shape, dt_ in (
            ("xT0", [M, S], F16),
            ("xT1", [M, S], F16),
            ("wqT", [M, NH * D], F16),
            ("wkT", [M, NH * D], F16),
            ("wvT", [M, NH * D], F16),
            ("packC", [128, S], F16),
            ("packS", [128, S], F16),
            ("tri", [128, 128], F16),
            ("ones", [128, 128], F16),
            ("bias4", [128, 1], F32),
        ):
            io[name] = nc.dram_tensor(name, shape, dt_, kind="ExternalInput").ap()
        io["outT"] = nc.dram_tensor(
            "outT", [NB * NH, 128, S], F32, kind="ExternalOutput"
        ).ap()
        with tile.TileContext(nc) as tc:
            build_attention(tc, io, S, M)
        nc.compile()
        _NC_CACHE["nc"] = nc
    return _NC_CACHE["nc"]


def kernel(x, Wq, Wk, Wv):
    x = np.asarray(x, dtype=np.float32)
    Wq = np.asarray(Wq, dtype=np.float32)
    Wk = np.asarray(Wk, dtype=np.float32)
    Wv = np.asarray(Wv, dtype=np.float32)

    nc = _get_nc()
    in_maps = [prep_core_inputs(x, Wq, Wk, Wv, c, S, M) for c in range(NCORES)]
    res = bass_utils.run_bass_kernel_spmd(nc, in_maps, core_ids=list(range(NCORES)))

    out = np.empty((NB, S, M), dtype=np.float32)
    for c in range(NCORES):
        outT = res.results[c]["outT"]
        for u in range(NB * NH):
            b, hl = u // NH, u % NH
            col = c * NH * D + hl * D
            out[b, :, col:col + D] = outT[u].T
    return out
